# revision 10
# baseline (speedup 1.0000x reference)
"""Cross-attention Trainium2 kernel (8 NeuronCores, data-parallel).

Problem: B=4, C=64, H=64, W=64.
  q = conv1x1(v1, wq, bq); k = conv1x1(v2, wk, bk); v = conv1x1(v2, wv, bv)
  tokens n = (c, h) pairs (N = C*H = 4096), feature dim = W = 64
  out = softmax(q @ k^T) @ v

Sharding: core i handles batch b = i//2 and the q-token half h in
[32*(i%2), 32*(i%2+1)).  Every core needs the full v2[b]; no collectives.

The per-core roofline is the ACT engine: softmax needs exp of NQ*NK =
2048*4096 elements and ACT runs a fixed 1 elem/lane/cycle @1.2GHz -> 64
activations of [128,1024] ~ 64us that nothing else can absorb.  The
schedule hides everything else under the exp stream:

  - scores computed TRANSPOSED (sT[j,i] = k_j . q_i, k-tokens on
    partitions); after exp the tile is exactly the stationary layout the
    P@V matmul needs.  No max subtraction (|s| <= ~74 fits fp32 exp);
    softmax denominator via ones-columns appended to V.
  - j-outer / i-inner main loop with two live i-window accumulators:
    each K/V projection+transpose chunk unlocks score pairs immediately,
    so the first exp fires a few us into the kernel instead of after
    full setup; remaining setup is drip-fed one small piece per unit
    into the PE queue and hides in the PE slack under the ACT pace.
  - q/k path in f32r end-to-end for precision (scores feed exp, so bf16
    q/k costs ~1e-2 rel err); x and wT are shipped as raw fp32 bytes
    into f32r tiles, avoiding the engine-rounding staging pass.  Host
    preps layout: ones row on x (bias via matmul), [w.T; bias] stacked.
  - 512-row moving dims everywhere keep f32r at 1 cycle/row.
  - ACT does nothing but exp once the main loop starts; psum reads are
    DVE-only (GpSimd cannot access PSUM), psum->sbuf same-dtype copies
    go to the otherwise idle DMA queues.
  - V carries a 64-wide ones block (lhsT free 128, same matmul cost) so
    the denominator lands on psum partitions 64..127: the epilogue is a
    64-lane reciprocal + one multiply + DMA, no transpose, no broadcast.
    Output leaves in [w, token] layout; host gather transposes back.
"""

import numpy as np

B, C, H, W = 4, 64, 64, 64
HH = H // 2            # h-rows per core (q-token half)
NQ = C * HH            # q tokens per core = 2048
NK = C * H             # k tokens = 4096
JB = NK // 128         # 32 j-blocks of 128 k-tokens
NP = JB // 2           # 16 row-packed j-block pairs
IP = 512               # i-span per accumulator window (4 windows)
NCORES = 8

_CACHE = {}


def _build_nc():
    from contextlib import ExitStack

    import concourse.bass as bass
    import concourse.tile as tile
    from concourse import bacc, mybir
    from concourse.bass import ts
    from concourse.masks import make_identity

    F32 = mybir.dt.float32
    F32R = mybir.dt.float32r
    BF16 = mybir.dt.bfloat16
    AF = mybir.ActivationFunctionType
    ALU = mybir.AluOpType

    nc = bacc.Bacc(trn_type="TRN2", target_bir_lowering=False)

    # host-prepped (raw fp32 bytes into f32r tiles):
    #   wx1 = [wTq | wTk | wTv | x1_aug]  (rows: 64 channels + ones row)
    #   x2  = x2_aug
    wx1_d = nc.declare_dram_parameter("wx1", [C + 1, 3 * C + HH * W], F32R, False)
    x2_d = nc.declare_dram_parameter("x2", [C + 1, H * W], F32R, False)
    # output in transposed [w, token] layout; host gather fixes it up
    out_d = nc.declare_dram_parameter("out", [W, NQ], F32, True)

    with ExitStack() as ctx:
        tc = ctx.enter_context(tile.TileContext(nc))
        cp = ctx.enter_context(tc.tile_pool(name="const", bufs=1))
        su = ctx.enter_context(tc.tile_pool(name="su", bufs=2, space="PSUM"))
        sp = ctx.enter_context(tc.tile_pool(name="sp", bufs=2, space="PSUM"))
        op = ctx.enter_context(tc.tile_pool(name="op", bufs=1, space="PSUM"))
        pp = ctx.enter_context(tc.tile_pool(name="pp", bufs=4))
        rp = ctx.enter_context(tc.tile_pool(name="rp", bufs=2))
        onp = ctx.enter_context(tc.tile_pool(name="onp", bufs=2))

        # ---- input DMAs: one per queue ----
        wx1 = cp.tile([C + 1, 3 * C + HH * W], F32R, tag="wx1")
        x2_sb = cp.tile([C + 1, H * W], F32R, tag="x2")
        nc.sync.dma_start(wx1[:, :], wx1_d[:, :])
        nc.scalar.dma_start(x2_sb[:, :], x2_d[:, :])
        wT = {n: wx1[:, ts(i, C)] for i, n in enumerate(("q", "k", "v"))}
        x1_sb = wx1[:, 3 * C : 3 * C + HH * W]

        identf = cp.tile([C, C], F32, tag="identf")
        make_identity(nc, identf[:, :])
        identr = cp.tile([C, C], F32R, tag="identr")
        nc.vector.tensor_copy(identr[:, :], identf[:, :])

        # prewarm the exp table set (emitted after the DMA issues so the
        # 1.3us table load doesn't delay them on the ACT queue)
        warm = cp.tile([128, 2], F32, tag="warm")
        nc.vector.memset(warm[:, :], 0.0)
        nc.scalar.activation(warm[:, 0:1], warm[:, 1:2], AF.Exp)

        # ---- persistent operand tiles ----
        Q_cm = cp.tile([C, HH * W], F32R, tag="Qcm")   # [c_out, (h, w)]
        K_cm = cp.tile([C, H * W], F32R, tag="Kcm")
        # qT2: [w, i] duplicated on both partition halves (rhs of scores)
        # kT2: [w, j] even j-blocks on partitions 0-63, odd on 64-127 (lhsT)
        qT2 = cp.tile([128, NQ], F32R, tag="qT2")
        kT2 = cp.tile([128, NK // 2], F32R, tag="kT2")
        # vf (128, JB, 128) bf16: partition p of block jb = v-token
        # (h = 2*jb + p//64, c = p%64); cols 0:64 = v features, cols
        # 64:128 = 1.0 so the P@V matmul lands the softmax denominator on
        # psum partitions 64:128 (moving-dim cost is unchanged)
        vf = cp.tile([128, JB, 128], BF16, tag="vf")
        nc.gpsimd.memset(vf[:, :, 64:128], 1.0)

        def project(dst, wname, x_sb, tg):
            ps = su.tile([C, 512], F32, tag="setup")
            nc.tensor.matmul(
                ps[:, :], lhsT=wT[wname], rhs=x_sb[:, ts(tg, 512)],
                start=True, stop=True,
            )
            nc.vector.tensor_copy(dst[:, ts(tg, 512)], ps[:, :])

        def q_tr(tg):
            # 8 h-blocks -> qT2 cols [512*tg, 512*(tg+1)), both halves
            ps = su.tile([64, 512], F32R, tag="setup")
            for hh in range(8):
                nc.tensor.transpose(
                    ps[:, ts(hh, 64)], Q_cm[:, ts(tg * 8 + hh, 64)], identr[:, :]
                )
            nc.vector.tensor_copy(qT2[0:64, ts(tg, 512)], ps[:, :])
            # second half is an SBUF->SBUF dup: free on the idle DMA queue
            nc.sync.dma_start(qT2[64:128, ts(tg, 512)], qT2[0:64, ts(tg, 512)])

        def k_tr(tg):
            # h in [8tg, 8tg+8) -> j-blocks [4tg, 4tg+4) -> pairs [2tg, 2tg+2)
            ps = su.tile([64, 512], F32R, tag="setup")
            for hh in range(8):
                nc.tensor.transpose(
                    ps[:, ts(hh, 64)], K_cm[:, ts(tg * 8 + hh, 64)], identr[:, :]
                )
            # cols = (hh, c) = (g2, hf, h2, c); kT2 col = pair*128 + h2*64 + c
            pv = ps[:, :].rearrange("p (g2 hf h2 c) -> p hf g2 h2 c", g2=2, hf=2, c=64)
            for hf in range(2):
                dst = kT2[64 * hf : 64 * (hf + 1), 2 * tg * 128 : 2 * tg * 128 + 256]
                nc.vector.tensor_copy(
                    dst.rearrange("p (g2 h2 c) -> p g2 h2 c", g2=2, c=64),
                    pv[:, hf, :, :, :],
                )

        def project_v(tg):
            # h in [8tg, 8tg+8) -> vf j-blocks [4tg, 4tg+4)
            ps = su.tile([C, 512], F32, tag="setup")
            nc.tensor.matmul(
                ps[:, :], lhsT=wT["v"], rhs=x2_sb[:, ts(tg, 512)],
                start=True, stop=True,
            )
            pv = ps[:, :].rearrange("p (jl h1 w) -> p h1 jl w", h1=2, w=W)
            for h1 in range(2):
                nc.vector.tensor_copy(
                    vf[64 * h1 : 64 * (h1 + 1), 4 * tg : 4 * tg + 4, 0:W],
                    pv[:, h1, :, :],
                )

        # ---- lead-in: just enough for pair 0 of both i-windows ----
        project(Q_cm, "q", x1_sb, 0)
        q_tr(0)
        project(Q_cm, "q", x1_sb, 1)
        q_tr(1)
        project(K_cm, "k", x2_sb, 0)
        k_tr(0)
        project_v(0)

        # remaining setup, drip-fed one piece per unit (each piece lands
        # several pairs before the units that consume it)
        pieces = [lambda: project(K_cm, "k", x2_sb, 1)]
        pieces.append(lambda: k_tr(1))
        pieces.append(lambda: project_v(1))
        pieces.append(lambda: project(Q_cm, "q", x1_sb, 2))
        pieces.append(lambda: project(Q_cm, "q", x1_sb, 3))
        for t in range(2, 8):
            pieces.append(lambda t=t: project(K_cm, "k", x2_sb, t))
            pieces.append(lambda t=t: k_tr(t))
            pieces.append(lambda t=t: project_v(t))
        pieces.append(lambda: q_tr(2))
        pieces.append(lambda: q_tr(3))
        pieces.reverse()  # pop() from the front

        def drain(acc, ih):
            rec = rp.tile([64, IP], F32, tag="rec")
            nc.vector.reciprocal(rec[:, :], acc[64:128, :])
            on = onp.tile([64, IP], F32, tag="on")
            nc.vector.scalar_tensor_tensor(
                on[:, :], acc[0:64, :], 1.0, rec[:, :], ALU.mult, ALU.mult
            )
            nc.sync.dma_start(out_d[:, ih * IP : (ih + 1) * IP], on[:, :])

        # ---- main loop: j-pairs outer, two i-windows inner ----
        for grp in range(2):
            accs = [
                op.tile([128, IP], F32, tag=f"acc{k}", name=f"acc{grp}_{k}")
                for k in range(2)
            ]
            for p in range(NP):
                for k in range(2):
                    ih = 2 * grp + k
                    sps = sp.tile([128, 2 * IP], F32, tag="sc")
                    for blk in range(2):
                        hf = 64 * blk
                        nc.tensor.matmul(
                            sps[:, ts(blk, IP)],
                            lhsT=kT2[hf : hf + 64, ts(p, 128)],
                            rhs=qT2[hf : hf + 64, ih * IP : (ih + 1) * IP],
                            start=True, stop=True,
                        )
                    pt = pp.tile([128, 2 * IP], BF16, tag="pt")
                    nc.scalar.activation(pt[:, :], sps[:, :], AF.Exp)
                    for blk in range(2):
                        jb = 2 * p + blk
                        nc.tensor.matmul(
                            accs[k][:, :],
                            lhsT=vf[:, jb, :],
                            rhs=pt[:, ts(blk, IP)],
                            start=(p == 0 and blk == 0),
                            stop=(p == NP - 1 and blk == 1),
                        )
                    if grp == 0 and pieces:
                        pieces.pop()()
                    if p == NP - 1:
                        drain(accs[k], ih)

    nc.compile()
    return nc


def _get_nc():
    if "nc" not in _CACHE:
        _CACHE["nc"] = _build_nc()
    return _CACHE["nc"]


def _in_maps(v1, v2, wq, bq, wk, bk, wv, bv):
    wTs = np.concatenate(
        [
            np.concatenate(
                [np.asarray(w, np.float32).T, np.asarray(b, np.float32).reshape(1, C)]
            )
            for w, b in ((wq, bq), (wk, bk), (wv, bv))
        ],
        axis=1,
    )  # [C+1, 3C]
    ones1 = np.ones((1, HH * W), np.float32)
    ones2 = np.ones((1, H * W), np.float32)
    maps = []
    for core in range(NCORES):
        b, half = divmod(core, 2)
        x1 = np.asarray(
            v1[b, :, half * HH : (half + 1) * HH, :], dtype=np.float32
        ).reshape(C, HH * W)
        x2 = np.asarray(v2[b], dtype=np.float32).reshape(C, H * W)
        maps.append({
            "wx1": np.ascontiguousarray(
                np.concatenate([wTs, np.concatenate([x1, ones1])], axis=1)
            ),
            "x2": np.ascontiguousarray(np.concatenate([x2, ones2])),
        })
    return maps


def _gather(results):
    out = np.zeros((B, C, H, W), dtype=np.float32)
    for core in range(NCORES):
        b, half = divmod(core, 2)
        # device out: [w, i] with token i = h_local*64 + c
        o = np.asarray(results[core]["out"], np.float32).reshape(W, HH, C)
        out[b, :, half * HH : (half + 1) * HH, :] = o.transpose(2, 1, 0)
    return out


def _run(trace=False, **inputs):
    from concourse.bass_utils import run_bass_kernel_spmd

    nc = _get_nc()
    maps = _in_maps(**inputs)
    res = run_bass_kernel_spmd(
        nc, maps, core_ids=list(range(NCORES)), trace=trace
    )
    return _gather(res.results), res


def kernel(**inputs):
    out, _ = _run(trace=False, **inputs)
    return out


# revision 11
# speedup vs baseline: 1.3382x; 1.3382x over previous
"""Cross-attention Trainium2 kernel (8 NeuronCores, data-parallel).

Problem: B=4, C=64, H=64, W=64.
  q = conv1x1(v1, wq, bq); k = conv1x1(v2, wk, bk); v = conv1x1(v2, wv, bv)
  tokens n = (c, h) pairs (N = C*H = 4096), feature dim = W = 64
  out = softmax(q @ k^T) @ v

Sharding: core i handles batch b = i//2 and the q-token half h in
[32*(i%2), 32*(i%2+1)).  Every core needs the full v2[b]; no collectives.

The per-core roofline is the ACT engine: softmax needs exp of NQ*NK =
2048*4096 elements and ACT runs a fixed 1 elem/lane/cycle @1.2GHz -> 64
activations of [128,1024] ~ 64us that nothing else can absorb.  The
schedule hides everything else under the exp stream:

  - scores computed TRANSPOSED (sT[j,i] = k_j . q_i, k-tokens on
    partitions); after exp the tile is exactly the stationary layout the
    P@V matmul needs.  No max subtraction (|s| <= ~74 fits fp32 exp);
    softmax denominator via ones-columns appended to V.
  - j-outer / i-inner main loop with two live i-window accumulators:
    each K/V projection+transpose chunk unlocks score pairs immediately,
    so the first exp fires a few us into the kernel instead of after
    full setup; remaining setup is drip-fed one small piece per unit
    into the PE queue and hides in the PE slack under the ACT pace.
  - q/k path in f32r end-to-end for precision (scores feed exp, so bf16
    q/k costs ~1e-2 rel err); x and wT are shipped as raw fp32 bytes
    into f32r tiles, avoiding the engine-rounding staging pass.  Host
    preps layout: ones row on x (bias via matmul), [w.T; bias] stacked.
  - 512-row moving dims everywhere keep f32r at 1 cycle/row.
  - ACT does nothing but exp once the main loop starts; psum reads are
    DVE-only (GpSimd cannot access PSUM), psum->sbuf same-dtype copies
    go to the otherwise idle DMA queues.
  - V carries a 64-wide ones block (lhsT free 128, same matmul cost) so
    the denominator lands on psum partitions 64..127: the epilogue is a
    64-lane reciprocal + one multiply + DMA, no transpose, no broadcast.
    Output leaves in [w, token] layout; host gather transposes back.
"""

import numpy as np

B, C, H, W = 4, 64, 64, 64
HH = H // 2            # h-rows per core (q-token half)
NQ = C * HH            # q tokens per core = 2048
NK = C * H             # k tokens = 4096
JB = NK // 128         # 32 j-blocks of 128 k-tokens
NP = JB // 2           # 16 row-packed j-block pairs
IP = 512               # i-span per accumulator window (4 windows)
NCORES = 8

_CACHE = {}


def _build_nc():
    from contextlib import ExitStack

    import concourse.bass as bass
    import concourse.tile as tile
    from concourse import bacc, mybir
    from concourse.bass import ts
    from concourse.masks import make_identity

    F32 = mybir.dt.float32
    F32R = mybir.dt.float32r
    BF16 = mybir.dt.bfloat16
    AF = mybir.ActivationFunctionType
    ALU = mybir.AluOpType

    nc = bacc.Bacc(trn_type="TRN2", target_bir_lowering=False)

    # host-prepped (raw fp32 bytes into f32r tiles):
    #   wx1 = [wTq | wTk | wTv | x1_aug]  (rows: 64 channels + ones row)
    #   x2  = x2_aug
    wx1_d = nc.declare_dram_parameter("wx1", [C + 1, 3 * C + HH * W], F32R, False)
    x2_d = nc.declare_dram_parameter("x2", [C + 1, H * W], F32R, False)
    # output in transposed [w, token] layout; host gather fixes it up
    out_d = nc.declare_dram_parameter("out", [W, NQ], F32, True)

    with ExitStack() as ctx:
        tc = ctx.enter_context(tile.TileContext(nc))
        cp = ctx.enter_context(tc.tile_pool(name="const", bufs=1))
        # one shared 3-deep PSUM ring (12KB) for score tiles AND setup
        # pieces: scores get >=2 slots of lookahead so the in-order PE
        # queue can run ahead of the exp stream, and setup pieces borrow
        # ring slots instead of their own banks; accs take the last 4KB
        sp = ctx.enter_context(tc.tile_pool(name="sp", bufs=3, space="PSUM"))
        op = ctx.enter_context(tc.tile_pool(name="op", bufs=1, space="PSUM"))
        pp = ctx.enter_context(tc.tile_pool(name="pp", bufs=6))
        rp = ctx.enter_context(tc.tile_pool(name="rp", bufs=2))
        onp = ctx.enter_context(tc.tile_pool(name="onp", bufs=2))

        # ---- input DMAs: one per queue ----
        wx1 = cp.tile([C + 1, 3 * C + HH * W], F32R, tag="wx1")
        x2_sb = cp.tile([C + 1, H * W], F32R, tag="x2")
        nc.sync.dma_start(wx1[:, :], wx1_d[:, :])
        nc.scalar.dma_start(x2_sb[:, :], x2_d[:, :])
        wT = {n: wx1[:, ts(i, C)] for i, n in enumerate(("q", "k", "v"))}
        x1_sb = wx1[:, 3 * C : 3 * C + HH * W]

        identf = cp.tile([C, C], F32, tag="identf")
        make_identity(nc, identf[:, :])
        identr = cp.tile([C, C], F32R, tag="identr")
        nc.vector.tensor_copy(identr[:, :], identf[:, :])

        # prewarm the exp table set (emitted after the DMA issues so the
        # 1.3us table load doesn't delay them on the ACT queue)
        warm = cp.tile([128, 2], F32, tag="warm")
        nc.vector.memset(warm[:, :], 0.0)
        nc.scalar.activation(warm[:, 0:1], warm[:, 1:2], AF.Exp)

        # ---- persistent operand tiles ----
        Q_cm = cp.tile([C, HH * W], F32R, tag="Qcm")   # [c_out, (h, w)]
        K_cm = cp.tile([C, H * W], F32R, tag="Kcm")
        # qT2: [w, i] duplicated on both partition halves (rhs of scores)
        # kT2: [w, j] even j-blocks on partitions 0-63, odd on 64-127 (lhsT)
        qT2 = cp.tile([128, NQ], F32R, tag="qT2")
        kT2 = cp.tile([128, NK // 2], F32R, tag="kT2")
        # vf (128, JB, 128) bf16: partition p of block jb = v-token
        # (h = 2*jb + p//64, c = p%64); cols 0:64 = v features, cols
        # 64:128 = 1.0 so the P@V matmul lands the softmax denominator on
        # psum partitions 64:128 (moving-dim cost is unchanged)
        vf = cp.tile([128, JB, 128], BF16, tag="vf")
        nc.gpsimd.memset(vf[:, :, 64:128], 1.0)

        def project(dst, wname, x_sb, tg):
            ps = sp.tile([C, 512], F32, tag="sc")
            nc.tensor.matmul(
                ps[:, :], lhsT=wT[wname], rhs=x_sb[:, ts(tg, 512)],
                start=True, stop=True,
            )
            nc.vector.tensor_copy(dst[:, ts(tg, 512)], ps[:, :])

        def q_tr(tg):
            # 8 h-blocks -> qT2 cols [512*tg, 512*(tg+1)), both halves
            ps = sp.tile([64, 512], F32R, tag="sc")
            for hh in range(8):
                nc.tensor.transpose(
                    ps[:, ts(hh, 64)], Q_cm[:, ts(tg * 8 + hh, 64)], identr[:, :]
                )
            nc.vector.tensor_copy(qT2[0:64, ts(tg, 512)], ps[:, :])
            # second half is an SBUF->SBUF dup: free on the idle DMA queue
            nc.sync.dma_start(qT2[64:128, ts(tg, 512)], qT2[0:64, ts(tg, 512)])

        def k_tr(tg):
            # h in [8tg, 8tg+8) -> j-blocks [4tg, 4tg+4) -> pairs [2tg, 2tg+2)
            ps = sp.tile([64, 512], F32R, tag="sc")
            for hh in range(8):
                nc.tensor.transpose(
                    ps[:, ts(hh, 64)], K_cm[:, ts(tg * 8 + hh, 64)], identr[:, :]
                )
            # cols = (hh, c) = (g2, hf, h2, c); kT2 col = pair*128 + h2*64 + c
            pv = ps[:, :].rearrange("p (g2 hf h2 c) -> p hf g2 h2 c", g2=2, hf=2, c=64)
            for hf in range(2):
                dst = kT2[64 * hf : 64 * (hf + 1), 2 * tg * 128 : 2 * tg * 128 + 256]
                nc.vector.tensor_copy(
                    dst.rearrange("p (g2 h2 c) -> p g2 h2 c", g2=2, c=64),
                    pv[:, hf, :, :, :],
                )

        def project_v(tg):
            # h in [8tg, 8tg+8) -> vf j-blocks [4tg, 4tg+4)
            ps = sp.tile([C, 512], F32, tag="sc")
            nc.tensor.matmul(
                ps[:, :], lhsT=wT["v"], rhs=x2_sb[:, ts(tg, 512)],
                start=True, stop=True,
            )
            pv = ps[:, :].rearrange("p (jl h1 w) -> p h1 jl w", h1=2, w=W)
            for h1 in range(2):
                nc.vector.tensor_copy(
                    vf[64 * h1 : 64 * (h1 + 1), 4 * tg : 4 * tg + 4, 0:W],
                    pv[:, h1, :, :],
                )

        # ---- lead-in: just enough for pair 0 of both i-windows ----
        project(Q_cm, "q", x1_sb, 0)
        q_tr(0)
        project(Q_cm, "q", x1_sb, 1)
        q_tr(1)
        project(K_cm, "k", x2_sb, 0)
        k_tr(0)
        project_v(0)

        # remaining setup, drip-fed one piece per unit (each piece lands
        # several pairs before the units that consume it)
        pieces = [lambda: project(K_cm, "k", x2_sb, 1)]
        pieces.append(lambda: k_tr(1))
        pieces.append(lambda: project_v(1))
        pieces.append(lambda: project(Q_cm, "q", x1_sb, 2))
        pieces.append(lambda: project(Q_cm, "q", x1_sb, 3))
        for t in range(2, 8):
            pieces.append(lambda t=t: project(K_cm, "k", x2_sb, t))
            pieces.append(lambda t=t: k_tr(t))
            pieces.append(lambda t=t: project_v(t))
        pieces.append(lambda: q_tr(2))
        pieces.append(lambda: q_tr(3))
        pieces.reverse()  # pop() from the front

        def drain(acc, ih):
            rec = rp.tile([64, IP], F32, tag="rec")
            nc.vector.reciprocal(rec[:, :], acc[64:128, :])
            on = onp.tile([64, IP], F32, tag="on")
            nc.vector.scalar_tensor_tensor(
                on[:, :], acc[0:64, :], 1.0, rec[:, :], ALU.mult, ALU.mult
            )
            nc.sync.dma_start(out_d[:, ih * IP : (ih + 1) * IP], on[:, :])

        # ---- main loop: j-pairs outer, two i-windows inner ----
        for grp in range(2):
            accs = [
                op.tile([128, IP], F32, tag=f"acc{k}", name=f"acc{grp}_{k}")
                for k in range(2)
            ]
            for p in range(NP):
                for k in range(2):
                    ih = 2 * grp + k
                    sps = sp.tile([128, 2 * IP], F32, tag="sc")
                    for blk in range(2):
                        hf = 64 * blk
                        nc.tensor.matmul(
                            sps[:, ts(blk, IP)],
                            lhsT=kT2[hf : hf + 64, ts(p, 128)],
                            rhs=qT2[hf : hf + 64, ih * IP : (ih + 1) * IP],
                            start=True, stop=True,
                        )
                    pt = pp.tile([128, 2 * IP], BF16, tag="pt")
                    nc.scalar.activation(pt[:, :], sps[:, :], AF.Exp)
                    for blk in range(2):
                        jb = 2 * p + blk
                        nc.tensor.matmul(
                            accs[k][:, :],
                            lhsT=vf[:, jb, :],
                            rhs=pt[:, ts(blk, IP)],
                            start=(p == 0 and blk == 0),
                            stop=(p == NP - 1 and blk == 1),
                        )
                    if grp == 0 and pieces:
                        pieces.pop()()
                    if p == NP - 1:
                        drain(accs[k], ih)

    nc.compile()
    return nc


def _get_nc():
    if "nc" not in _CACHE:
        _CACHE["nc"] = _build_nc()
    return _CACHE["nc"]


def _in_maps(v1, v2, wq, bq, wk, bk, wv, bv):
    wTs = np.concatenate(
        [
            np.concatenate(
                [np.asarray(w, np.float32).T, np.asarray(b, np.float32).reshape(1, C)]
            )
            for w, b in ((wq, bq), (wk, bk), (wv, bv))
        ],
        axis=1,
    )  # [C+1, 3C]
    ones1 = np.ones((1, HH * W), np.float32)
    ones2 = np.ones((1, H * W), np.float32)
    maps = []
    for core in range(NCORES):
        b, half = divmod(core, 2)
        x1 = np.asarray(
            v1[b, :, half * HH : (half + 1) * HH, :], dtype=np.float32
        ).reshape(C, HH * W)
        x2 = np.asarray(v2[b], dtype=np.float32).reshape(C, H * W)
        maps.append({
            "wx1": np.ascontiguousarray(
                np.concatenate([wTs, np.concatenate([x1, ones1])], axis=1)
            ),
            "x2": np.ascontiguousarray(np.concatenate([x2, ones2])),
        })
    return maps


def _gather(results):
    out = np.zeros((B, C, H, W), dtype=np.float32)
    for core in range(NCORES):
        b, half = divmod(core, 2)
        # device out: [w, i] with token i = h_local*64 + c
        o = np.asarray(results[core]["out"], np.float32).reshape(W, HH, C)
        out[b, :, half * HH : (half + 1) * HH, :] = o.transpose(2, 1, 0)
    return out


def _run(trace=False, **inputs):
    from concourse.bass_utils import run_bass_kernel_spmd

    nc = _get_nc()
    maps = _in_maps(**inputs)
    res = run_bass_kernel_spmd(
        nc, maps, core_ids=list(range(NCORES)), trace=trace
    )
    return _gather(res.results), res


def kernel(**inputs):
    out, _ = _run(trace=False, **inputs)
    return out
